# revision 8
# baseline (speedup 1.0000x reference)
"""Trainium2 Bass kernel for AttentionNet (conv frontend + MHA + readout).

Strategy: pure data-parallel over batch (64 samples -> 8 cores x 8), with an
AllReduce for the BatchNorm batch statistics. All heavy matmuls in fp32r.

Per-core pipeline:
  conv1d (im2col matmul, K=52) -> maxpool(4) + BN partial stats
  -> AllReduce stats -> BN scale+ReLU -> per-head QKV -> scores^T (m-major)
  -> exp(s/8) * exp(-bias) (Toeplitz bias folded into a precomputed factor)
  -> unnormalized attn @ [V | ones] (row-sums come out duplicated across 64
     partitions = free reciprocal broadcast) -> normalize -> Wm + ReLU with
  accum_out readout-sum -> standardize -> Wo.
"""

import sys

for p in ("/opt/trn_rl_repo", "/opt/pypackages"):
    if p not in sys.path:
        sys.path.insert(0, p)

import numpy as np

import concourse.bass as bass
import concourse.bacc as bacc
import concourse.tile as tile
import concourse.mybir as mybir
from concourse import bass_utils

F32 = mybir.dt.float32
F32R = mybir.dt.float32r
AF = mybir.ActivationFunctionType
AX = mybir.AxisListType

N_CORES = 8
B, L, C_IN = 64, 2000, 4
F, KW, PAD = 256, 13, 6
POOL = 4
H, DH = 8, 64
DM, NCLS = 100, 2
NOUT = L // POOL  # 500
BPC = B // N_CORES  # 8 samples per core
LP = L + 2 * PAD  # 2012
KC = F // 128  # 2 contraction chunks of 128
MCH = [128, 128, 128, 116]  # NOUT split into partition chunks
BN_N = float(B * L)  # batchnorm reduction count


def _build_program():
    nc = bacc.Bacc("TRN2", target_bir_lowering=False, debug=False,
                   num_devices=N_CORES)

    dram = {}

    def din(name, shape, dt=F32R):
        dram[name] = nc.dram_tensor(name, list(shape), dt, kind="ExternalInput").ap()
        return dram[name]

    xpad = din("xpad", [BPC, C_IN, LP])          # transposed+padded x shard
    wc = din("wc", [C_IN * KW, F])               # conv lhsT (52, 256)
    wq = din("wq", [128, KC * H * DH])           # [p, kc*512 + h*64 + d]
    wk = din("wk", [128, KC * H * DH])
    wv = din("wv", [128, KC * H * DH])
    wm = din("wm", [128, 4 * DM])                # [p, c*100 + j]
    wo = din("wo", [DM + 1, NCLS])               # [Wo; bo]
    emat = din("emat", [NOUT, NOUT], F32)        # exp(-bias[n,m]) (symmetric)
    bqk = din("bqk", [128, H], F32)              # rows 0:64 bq, 64:128 bk
    bm_eff = din("bm_eff", [DM], F32)            # bm + Wm^T contribution of bv
    gam2 = din("gam2", [128, 2], F32)
    bet2 = din("bet2", [128, 2], F32)
    out = nc.dram_tensor("out", [BPC, NCLS], F32, kind="ExternalOutput").ap()

    with tile.TileContext(nc) as tc:
        _emit(tc, dram, out)
    nc.compile()
    return nc


def _emit(tc, d, out):
    nc = tc.nc
    from contextlib import ExitStack

    ctx = ExitStack()
    with ctx:
        cst = ctx.enter_context(tc.tile_pool(name="cst", bufs=1))

        # ---- constants into SBUF ----
        wc_sb = cst.tile([C_IN * KW, F], F32R)
        nc.sync.dma_start(wc_sb[:], d["wc"][:])
        wq_sb = cst.tile([128, KC * H * DH], F32R)
        nc.sync.dma_start(wq_sb[:], d["wq"][:])
        wk_sb = cst.tile([128, KC * H * DH], F32R)
        nc.sync.dma_start(wk_sb[:], d["wk"][:])
        wv_sb = cst.tile([128, KC * H * DH], F32R)
        nc.sync.dma_start(wv_sb[:], d["wv"][:])
        wm_sb = cst.tile([128, 4 * DM], F32R)
        nc.sync.dma_start(wm_sb[:], d["wm"][:])
        wo_sb = cst.tile([DM + 1, NCLS], F32R)
        nc.sync.dma_start(wo_sb[:], d["wo"][:])
        bqk_sb = cst.tile([128, H], F32)
        nc.sync.dma_start(bqk_sb[:], d["bqk"][:])
        bm_sb = cst.tile([DM, 1], F32)
        nc.sync.dma_start(bm_sb[:], d["bm_eff"][:])
        gam_sb = cst.tile([128, 2], F32)
        nc.sync.dma_start(gam_sb[:], d["gam2"][:])
        bet_sb = cst.tile([128, 2], F32)
        nc.sync.dma_start(bet_sb[:], d["bet2"][:])
        e_sb = []
        for mc in range(4):
            m0 = sum(MCH[:mc])
            t = cst.tile([MCH[mc], NOUT], F32, tag=f"e{mc}", name=f"e{mc}")
            nc.sync.dma_start(t[:], d["emat"][m0:m0 + MCH[mc], :])
            e_sb.append(t)

        ym = [cst.tile([128, BPC * NOUT], F32, tag=f"ym{i}", name=f"ym{i}") for i in range(2)]
        ssum = [cst.tile([128, BPC * 4], F32, tag=f"ssum{i}", name=f"ssum{i}") for i in range(2)]
        ssq = [cst.tile([128, BPC * 4], F32, tag=f"ssq{i}", name=f"ssq{i}") for i in range(2)]
        featn = cst.tile([DM, BPC], F32)
        ones_k = cst.tile([DM, 1], F32)
        nc.gpsimd.memset(ones_k[:], 1.0)
        ones_p = cst.tile([1, DM], F32)
        nc.gpsimd.memset(ones_p[:], 1.0)

        # ---- phase A: conv + maxpool + BN partial stats ----
        with tc.tile_pool(name="imp", bufs=2) as imp, \
             tc.tile_pool(name="cps", bufs=4, space="PSUM") as cps, \
             tc.tile_pool(name="sqp", bufs=2) as sqp:
            for s in range(BPC):
                imt = imp.tile([C_IN * KW, L], F32R)
                xs = d["xpad"][s]  # [4, 2012]
                src = bass.AP(xs.tensor, xs.offset, [[LP, C_IN], [1, KW], [1, L]])
                nc.sync.dma_start(imt[:], src)
                for fh in range(2):
                    for c in range(4):
                        ps = cps.tile([128, NOUT], F32, tag="cps")
                        nc.tensor.matmul(
                            ps[:], wc_sb[:, fh * 128:(fh + 1) * 128],
                            imt[:, c * NOUT:(c + 1) * NOUT],
                            start=True, stop=True)
                        nc.vector.reduce_max(
                            ym[fh][:, s * NOUT + c * 125: s * NOUT + (c + 1) * 125],
                            ps[:].rearrange("p (a b) -> p a b", b=POOL),
                            axis=AX.X)
                        nc.vector.reduce_sum(
                            ssum[fh][:, s * 4 + c: s * 4 + c + 1], ps[:], axis=AX.X)
                        sqt = sqp.tile([128, NOUT], F32)
                        nc.scalar.activation(
                            sqt[:], ps[:], AF.Square,
                            accum_out=ssq[fh][:, s * 4 + c: s * 4 + c + 1])

        # ---- BN stats AllReduce + scale/bias ----
        bnp = cst.tile([128, 4], F32)
        for fh in range(2):
            nc.vector.reduce_sum(bnp[:, fh:fh + 1], ssum[fh][:], axis=AX.X)
            nc.vector.reduce_sum(bnp[:, 2 + fh:3 + fh], ssq[fh][:], axis=AX.X)
        with tc.tile_pool(name="drp", bufs=1, space="DRAM") as drp:
            bn_in = drp.tile([128, 4], F32)
            bn_out = drp.tile([128, 4], F32)
            nc.sync.dma_start(bn_in[:], bnp[:])
            nc.gpsimd.collective_compute(
                "AllReduce", mybir.AluOpType.add,
                replica_groups=[list(range(N_CORES))],
                ins=[bn_in[:].opt()], outs=[bn_out[:].opt()])
            stg = cst.tile([128, 4], F32)
            nc.sync.dma_start(stg[:], bn_out[:])

        mu = cst.tile([128, 2], F32)
        nc.vector.tensor_scalar_mul(mu[:], stg[:, 0:2], 1.0 / BN_N)
        ex2 = cst.tile([128, 2], F32)
        nc.vector.tensor_scalar_mul(ex2[:], stg[:, 2:4], 1.0 / BN_N)
        var = cst.tile([128, 2], F32)
        nc.vector.tensor_mul(var[:], mu[:], mu[:])
        nc.vector.tensor_sub(var[:], ex2[:], var[:])
        epst = cst.tile([128, 1], F32)
        nc.gpsimd.memset(epst[:], 1e-5)
        sd = cst.tile([128, 2], F32)
        nc.scalar.activation(sd[:], var[:], AF.Sqrt, bias=epst[:])
        rstd = cst.tile([128, 2], F32)
        nc.vector.reciprocal(rstd[:], sd[:])
        scl = cst.tile([128, 2], F32)
        nc.vector.tensor_mul(scl[:], rstd[:], gam_sb[:])
        bia = cst.tile([128, 2], F32)
        nc.vector.tensor_mul(bia[:], mu[:], scl[:])
        nc.vector.tensor_sub(bia[:], bet_sb[:], bia[:])

        # ---- phase C: attention per sample ----
        with tc.tile_pool(name="htp", bufs=4) as htp, \
             tc.tile_pool(name="vtp", bufs=8) as vtp, \
             tc.tile_pool(name="qtp", bufs=3) as qtp, \
             tc.tile_pool(name="ptp", bufs=4) as ptp, \
             tc.tile_pool(name="rbp", bufs=2) as rbp, \
             tc.tile_pool(name="ocp", bufs=8) as ocp, \
             tc.tile_pool(name="msp", bufs=2) as msp, \
             tc.tile_pool(name="vpp", bufs=1, space="PSUM") as vpp, \
             tc.tile_pool(name="qkp", bufs=1, space="PSUM") as qkp, \
             tc.tile_pool(name="scp", bufs=2, space="PSUM") as scp, \
             tc.tile_pool(name="opp", bufs=2, space="PSUM") as opp, \
             tc.tile_pool(name="mpp", bufs=1, space="PSUM") as mpp:
            for s in range(BPC):
                ht = []
                for fh in range(2):
                    t = htp.tile([128, NOUT], F32R, tag="ht", name="ht")
                    nc.scalar.activation(
                        t[:], ym[fh][:, s * NOUT:(s + 1) * NOUT], AF.Relu,
                        bias=bia[:, fh:fh + 1], scale=scl[:, fh:fh + 1])
                    ht.append(t)

                # V for all heads, layout [m, 8*(64 v | 64 ones)]
                vts = []
                for mc in range(4):
                    vt = vtp.tile([128, H * 128], F32R, tag="vt", name="vt")
                    ones_v = vt[:].rearrange("p (h x) -> p h x", x=128)[:, :, DH:128]
                    nc.vector.tensor_scalar(ones_v, ones_v, 0.0, 1.0,
                                            op0=mybir.AluOpType.mult,
                                            op1=mybir.AluOpType.add)
                    vp = vpp.tile([128, H * DH], F32)
                    m0, msz = sum(MCH[:mc]), MCH[mc]
                    for kc in range(KC):
                        nc.tensor.matmul(
                            vp[0:msz, :], ht[kc][:, m0:m0 + msz],
                            wv_sb[:, kc * 512:(kc + 1) * 512],
                            start=(kc == 0), stop=(kc == KC - 1))
                    nc.vector.tensor_copy(
                        vt[0:msz].rearrange("p (h x) -> p h x", x=128)[:, :, 0:DH],
                        vp[0:msz].rearrange("p (h x) -> p h x", x=DH))
                    vts.append(vt)

                ocs = [ocp.tile([128, NOUT], F32R, tag="oc", name="oc") for _ in range(4)]
                for h in range(H):
                    qps = qkp.tile([64, NOUT], F32, tag="qps", name="qps")
                    kps = qkp.tile([64, NOUT], F32, tag="kps", name="kps")
                    for kc in range(KC):
                        nc.tensor.matmul(
                            qps[:],
                            wq_sb[:, kc * 512 + h * DH: kc * 512 + (h + 1) * DH],
                            ht[kc][:], start=(kc == 0), stop=(kc == KC - 1))
                    for kc in range(KC):
                        nc.tensor.matmul(
                            kps[:],
                            wk_sb[:, kc * 512 + h * DH: kc * 512 + (h + 1) * DH],
                            ht[kc][:], start=(kc == 0), stop=(kc == KC - 1))
                    qt = qtp.tile([64, NOUT], F32R, tag="qt", name="qt")
                    kt = qtp.tile([64, NOUT], F32R, tag="kt", name="kt")
                    if h % 2 == 0:
                        nc.scalar.activation(qt[:], qps[:], AF.Identity,
                                             bias=bqk_sb[0:64, h:h + 1])
                        nc.scalar.activation(kt[:], kps[:], AF.Identity,
                                             bias=bqk_sb[64:128, h:h + 1])
                    else:
                        nc.vector.tensor_scalar_add(qt[:], qps[:],
                                                    bqk_sb[0:64, h:h + 1])
                        nc.vector.tensor_scalar_add(kt[:], kps[:],
                                                    bqk_sb[64:128, h:h + 1])

                    op = opp.tile([128, NOUT], F32, tag="op")
                    for mc in range(4):
                        m0, msz = sum(MCH[:mc]), MCH[mc]
                        sc = scp.tile([128, NOUT], F32, tag="sc")
                        nc.tensor.matmul(sc[0:msz, :], kt[:, m0:m0 + msz],
                                         qt[:], start=True, stop=True)
                        pt = ptp.tile([128, NOUT], F32R, tag="pt")
                        nc.scalar.activation(pt[0:msz, :], sc[0:msz, :], AF.Exp,
                                             scale=1.0 / 8.0)
                        eng = nc.vector if mc % 2 == 0 else nc.gpsimd
                        eng.tensor_mul(pt[0:msz, :], pt[0:msz, :], e_sb[mc][0:msz, :])
                        nc.tensor.matmul(op[:], vts[mc][0:msz, h * 128:(h + 1) * 128],
                                         pt[0:msz, :], start=(mc == 0), stop=(mc == 3))
                    rb = rbp.tile([64, NOUT], F32, tag="rb")
                    nc.vector.reciprocal(rb[:], op[64:128, :])
                    nc.vector.tensor_mul(
                        ocs[h // 2][(h % 2) * 64:(h % 2) * 64 + 64, :],
                        op[0:64, :], rb[:])

                mp = mpp.tile([DM, NOUT], F32)
                for c in range(4):
                    nc.tensor.matmul(mp[:], wm_sb[:, c * DM:(c + 1) * DM], ocs[c][:],
                                     start=(c == 0), stop=(c == 3))
                ms = msp.tile([DM, NOUT], F32)
                nc.scalar.activation(ms[:], mp[:], AF.Relu, bias=bm_sb[:],
                                     accum_out=featn[:, s:s + 1])

        # ---- final: standardize + Wo ----
        with tc.tile_pool(name="fsp", bufs=1) as fsp, \
             tc.tile_pool(name="fpp", bufs=1, space="PSUM") as fpp:
            fsq = fsp.tile([DM, BPC], F32)
            nc.scalar.activation(fsq[:], featn[:], AF.Square)
            cs = fpp.tile([1, 2 * BPC], F32, tag="cs")
            nc.tensor.matmul(cs[:, 0:BPC], ones_k[:], featn[:], start=True, stop=True)
            nc.tensor.matmul(cs[:, BPC:2 * BPC], ones_k[:], fsq[:], start=True,
                             stop=True)
            st = fsp.tile([1, 4 * BPC], F32)
            nc.vector.tensor_scalar_mul(st[:, 0:BPC], cs[:, 0:BPC], 1.0 / DM)
            nc.vector.tensor_scalar_mul(st[:, BPC:2 * BPC], cs[:, BPC:2 * BPC],
                                        1.0 / DM)
            nc.vector.tensor_mul(st[:, 2 * BPC:3 * BPC], st[:, 0:BPC], st[:, 0:BPC])
            nc.vector.tensor_sub(st[:, 3 * BPC:4 * BPC], st[:, BPC:2 * BPC],
                                 st[:, 2 * BPC:3 * BPC])
            sdt = fsp.tile([1, BPC], F32, tag="sdt")
            nc.scalar.activation(sdt[:], st[:, 3 * BPC:4 * BPC], AF.Sqrt)
            nc.vector.tensor_scalar_add(sdt[:], sdt[:], 1e-6)
            rsd = fsp.tile([1, BPC], F32, tag="rsd")
            nc.vector.reciprocal(rsd[:], sdt[:])
            bcm = fpp.tile([DM, BPC], F32, tag="bcm")
            nc.tensor.matmul(bcm[:], ones_p[:], st[:, 0:BPC], start=True, stop=True)
            bcr = fpp.tile([DM, BPC], F32, tag="bcr")
            nc.tensor.matmul(bcr[:], ones_p[:], rsd[:], start=True, stop=True)
            fc = fsp.tile([DM, BPC], F32, tag="fc")
            nc.vector.tensor_sub(fc[:], featn[:], bcm[:])
            fcn = fsp.tile([DM + 1, BPC], F32R, tag="fcn")
            nc.vector.tensor_scalar(fcn[:], fcn[:], 0.0, 1.0,
                                    op0=mybir.AluOpType.mult,
                                    op1=mybir.AluOpType.add)
            nc.vector.tensor_mul(fcn[0:DM, :], fc[:], bcr[:])
            fo = fpp.tile([BPC, NCLS], F32, tag="fo")
            nc.tensor.matmul(fo[:], fcn[:], wo_sb[:], start=True, stop=True)
            osb = fsp.tile([BPC, NCLS], F32, tag="osb")
            nc.vector.tensor_copy(osb[:], fo[:])
            nc.sync.dma_start(out[:], osb[:])


_NC_CACHE = None


def _get_program():
    global _NC_CACHE
    if _NC_CACHE is None:
        _NC_CACHE = _build_program()
    return _NC_CACHE


def _prep_inputs(x, conv_w, bn_gamma, bn_beta, Wq, bq, Wk, bk, Wv, bv, Wm, bm,
                 Wo, bo):
    f32 = np.float32
    x = np.asarray(x, f32)
    # [B, L, C] -> [B, C, L] padded
    xpad = np.zeros((B, C_IN, LP), f32)
    xpad[:, :, PAD:PAD + L] = np.transpose(x, (0, 2, 1))
    # conv lhsT [c*13+k, f]
    wc = np.ascontiguousarray(
        np.transpose(np.asarray(conv_w, f32), (1, 2, 0)).reshape(C_IN * KW, F))

    def proj_layout(W):  # [H, F, DH] -> [128, kc*512 + h*64 + d]
        W = np.asarray(W, f32)
        o = np.transpose(W.reshape(H, KC, 128, DH), (2, 1, 0, 3))  # p,kc,h,d
        return np.ascontiguousarray(o.reshape(128, KC * H * DH))

    wqh, wkh, wvh = proj_layout(Wq), proj_layout(Wk), proj_layout(Wv)
    wmh = np.ascontiguousarray(
        np.transpose(np.asarray(Wm, f32).reshape(4, 128, DM), (1, 0, 2))
        .reshape(128, 4 * DM))
    woh = np.concatenate([np.asarray(Wo, f32),
                          np.asarray(bo, f32)[None, :]], axis=0)
    bqkh = np.concatenate([np.asarray(bq, f32).T, np.asarray(bk, f32).T],
                          axis=0)  # [128, 8]
    bv_f = np.asarray(bv, f32).reshape(H * DH)
    bmh = np.asarray(bm, f32) + bv_f @ np.asarray(Wm, f32)
    gam2 = np.ascontiguousarray(np.asarray(bn_gamma, f32).reshape(2, 128).T)
    bet2 = np.ascontiguousarray(np.asarray(bn_beta, f32).reshape(2, 128).T)
    # Toeplitz bias factor: bias[n,m] = (4|n-m|+3)/1999 -> emat = exp(-bias)
    idx = np.arange(NOUT)
    bias = (4.0 * np.abs(idx[:, None] - idx[None, :]) + 3.0) / float(L - 1)
    emat = np.exp(-bias).astype(f32)

    shared = dict(wc=wc, wq=wqh, wk=wkh, wv=wvh, wm=wmh, wo=woh, bqk=bqkh,
                  bm_eff=bmh, gam2=gam2, bet2=bet2, emat=emat)
    in_maps = []
    for c in range(N_CORES):
        m = dict(shared)
        m["xpad"] = np.ascontiguousarray(xpad[c * BPC:(c + 1) * BPC])
        in_maps.append(m)
    return in_maps


def kernel(**inputs):
    in_maps = _prep_inputs(**inputs)
    nc = _get_program()
    res = bass_utils.run_bass_kernel_spmd(nc, in_maps, list(range(N_CORES)))
    return np.concatenate([res.results[i]["out"] for i in range(N_CORES)],
                          axis=0).astype(np.float32)


if __name__ == "__main__":
    rng = np.random.default_rng(0)
    dummy = {
        "x": rng.standard_normal((B, L, C_IN)).astype(np.float32),
        "conv_w": (rng.standard_normal((F, C_IN, KW)) / np.sqrt(C_IN * KW)).astype(np.float32),
        "bn_gamma": np.ones(F, np.float32),
        "bn_beta": np.zeros(F, np.float32),
        "Wq": (rng.standard_normal((H, F, DH)) / 16).astype(np.float32),
        "bq": np.zeros((H, DH), np.float32),
        "Wk": (rng.standard_normal((H, F, DH)) / 16).astype(np.float32),
        "bk": np.zeros((H, DH), np.float32),
        "Wv": (rng.standard_normal((H, F, DH)) / 16).astype(np.float32),
        "bv": np.zeros((H, DH), np.float32),
        "Wm": (rng.standard_normal((H * DH, DM)) / np.sqrt(H * DH)).astype(np.float32),
        "bm": np.zeros(DM, np.float32),
        "Wo": (rng.standard_normal((DM, NCLS)) / 10).astype(np.float32),
        "bo": np.zeros(NCLS, np.float32),
    }
    print(kernel(**dummy)[:4])
